# revision 44
# baseline (speedup 1.0000x reference)
"""Multi-head self-attention (B=4, S=2048, D=1024, H=16) on 8 NeuronCores.

Sharding: core c handles batch b=c//2 and head-half h0=(c%2)*8 (8 of 16 heads).
Each core computes q/k/v projections for its heads, full attention, and a
partial output projection over its 512-wide slice of the concat dim.
Host sums the two partial outputs per batch and adds bo + Wo@bv (the value
bias commutes through softmax since probabilities sum to 1).

v2: scoresT stays [j, i]; the attention-apply matmuls are reoriented to
out[i, t] (lhsT = exp-scores tile, rhs = v plus a ones column for the softmax
denominator), cutting their PE cost ~2x (free dim 65 instead of 512).  Both
heads of a pair run as single 64-matmul PSUM accumulation groups (one bank
each; disjoint [128,65] sub-tiles are first-touch-zeroed inside the group).
z is normalized on DVE into [i, t] tiles and transposed back to [t, i] with
the DMA XBAR (free for PE/DVE).

v3: two changes attack the two busiest engines.
 (a) PE: the score matmuls run in fp8e4 DoubleRow (0.5 cyc/row instead of
     1.0).  q/k are quantized to fp8 at projection-evac time into a DR
     layout [128p = 4 heads x 32 t_lo, group, 2 t_hi, S]; the host permutes
     Wq/Wk rows per pair to [h0 t<32 | h1 t<32 | h0 t>=32 | h1 t>=32] so the
     evac is two lane-shifted [64, 512] copies per tile.
 (b) Act: exp was the top bottleneck (256 instrs x ~1.04us).  A fraction of
     the exp tiles moves to DVE and Pool via a bf16 Schraudolph bit-trick:
     i16 = trunc(A*s + B) stored as int16 equals the bit pattern of
     bf16(exp(s*0.125)) up to a ~1.5% rms mantissa scallop, which the 2e-2
     accuracy budget absorbs.  One tensor_scalar per tile on either engine.
"""

import numpy as np
import ml_dtypes

from contextlib import ExitStack

import concourse.bacc as bacc
import concourse.bass as bass
import concourse.mybir as mybir
import concourse.tile as tile
from concourse.bass_utils import run_bass_kernel_spmd

BF16 = ml_dtypes.bfloat16
F8 = ml_dtypes.float8_e4m3

B, S, D, H, T = 4, 2048, 1024, 16, 64
HL = 8            # heads per core
DL = HL * T       # 512: local slice of concat dim
NC = 8            # cores
NPAIR = 4         # head pairs per core
NSB = 4           # 512-wide s/i blocks
NJB = 16          # 128-wide j blocks
NKC = 8           # 128-wide contraction chunks of D

f32 = mybir.dt.float32
bf16 = mybir.dt.bfloat16
i16 = mybir.dt.int16

# bf16 Schraudolph exp: trunc(EXP_A*(s/8) + EXP_B) as int16 == bf16 bits of
# ~exp(s/8).  EXP_A = 2^7*log2(e) (bf16 has a 7-bit mantissa); EXP_B is
# 127*2^7 minus a magic calibrated on-device so the mean ratio vs exp() is
# 1.0 (the scallop must not bias tiles against exact-exp tiles in the same
# softmax row).  Score scale 1/8 is folded into EXP_A.
EXP_A = float(np.float32(128.0 * 1.4426950408889634 * 0.125))
EXP_B = float(np.float32(16248.67))

# pair-local W row permutation: psum rows [0:64] = both heads' t_lo (t<32),
# rows [64:128] = both heads' t_hi, so the two fp8 evac copies are clean
# [64, 512] lane-shifts.
T_PERM = np.concatenate([np.arange(0, 32), np.arange(64, 96),
                         np.arange(32, 64), np.arange(96, 128)])

_STATE = {}


def _build():
    nc = bacc.Bacc("TRN2", target_bir_lowering=False, debug=False, num_devices=NC)

    fp8 = mybir.dt.float8e4
    xqd, xkd, xvd, wqd, wkd, wvd = {}, {}, {}, {}, {}, {}
    for part in ("hi", "lo"):
        xqd[part] = nc.dram_tensor(f"xq_{part}", [D, S], fp8,
                                   kind="ExternalInput").ap()
        xkd[part] = nc.dram_tensor(f"xk_{part}", [D, S], fp8,
                                   kind="ExternalInput").ap()
        xvd[part] = nc.dram_tensor(f"xv_{part}", [D, S], fp8,
                                   kind="ExternalInput").ap()
        wqd[part] = nc.dram_tensor(f"wq_{part}", [NPAIR, 128, NKC, 128],
                                   fp8, kind="ExternalInput").ap()
        wkd[part] = nc.dram_tensor(f"wk_{part}", [NPAIR, 128, NKC, 128],
                                   fp8, kind="ExternalInput").ap()
        wvd[part] = nc.dram_tensor(f"wv_{part}", [D, DL], fp8,
                                   kind="ExternalInput").ap()
    bq2 = nc.dram_tensor("bq2", [128, NPAIR], f32, kind="ExternalInput").ap()
    bk2 = nc.dram_tensor("bk2", [128, NPAIR], f32, kind="ExternalInput").ap()
    wo = nc.dram_tensor("wo", [DL, D], bf16, kind="ExternalInput").ap()
    out = nc.dram_tensor("out", [S, D], bf16, kind="ExternalOutput").ap()

    with tile.TileContext(nc) as tc:
        with ExitStack() as octx:
            persist = octx.enter_context(tc.tile_pool(name="persist", bufs=1))
            # fp8 DR layout: [p = 64*(pair%2) + 32*head + t_lo, pair//2,
            #                 t_hi, s]; score contraction t = 32*t_hi + t_lo
            qT = persist.tile([128, 2, 2, S], fp8, tag="qT")
            kT = persist.tile([128, 2, 2, S], fp8, tag="kT")
            # v_sb[:, jb, h, 0:64] = v[j, t]; [..., 64] = 1.0 (denominator col)
            v_sb = persist.tile([128, NJB, HL, 65], bf16, tag="v_sb")
            zT = persist.tile([128, NPAIR, S], bf16, tag="zT")
            bq_sb = persist.tile([128, NPAIR], f32, tag="bq_sb")
            bk_sb = persist.tile([128, NPAIR], f32, tag="bk_sb")

            # streamed x slabs [128, NKC, 512]
            xspool = octx.enter_context(tc.tile_pool(name="xsp", bufs=4))
            wpool = octx.enter_context(tc.tile_pool(name="wp", bufs=3))
            # PSUM (8 banks): sc [128,2,512] x2 bufs = 4 banks,
            # ctx [128,4,65] x2 = 2 banks, gemm [128,512] x2 = 2 banks
            psum = octx.enter_context(tc.tile_pool(name="psum", bufs=2,
                                                   space="PSUM"))
            epool = octx.enter_context(tc.tile_pool(name="ep", bufs=20))
            zpool = octx.enter_context(tc.tile_pool(name="zp", bufs=2))
            opool = octx.enter_context(tc.tile_pool(name="op", bufs=3))

            # dram views [128, chunk, cols]
            xkr = {p: xkd[p].rearrange("(c p) s -> p c s", p=128)
                   for p in ("hi", "lo")}
            xqr = {p: xqd[p].rearrange("(c p) s -> p c s", p=128)
                   for p in ("hi", "lo")}
            xvr = {p: xvd[p].rearrange("(c p) s -> p c s", p=128)
                   for p in ("hi", "lo")}
            wvr = {p: wvd[p].rearrange("(c p) d -> p c d", p=128)
                   for p in ("hi", "lo")}

            nc.vector.memset(v_sb[:, :, :, 64:65], 1.0)

            def warm(n, big=False):
                # dummy matmuls on the ones column: keep the PE p-state hot
                # across known stalls (cold prologue, tail transpose wait).
                if big:
                    wt = psum.tile([128, 512], f32, tag="sc0", name="warm")
                else:
                    wt = psum.tile([128, 512], f32, tag="gemm", name="warm")
                rhs = v_sb[:, 0, 0:7, 0:65] if big else v_sb[:, :, :, 64:65]
                nf = 455 if big else 128
                for _ in range(n):
                    nc.tensor.matmul(
                        out=wt[0:1, 0:nf],
                        lhsT=v_sb[:, 0, 0, 64:65],
                        rhs=rhs,
                        start=True, stop=True,
                    )

            xq_sb, xk_sb, xv_sb = {}, {}, {}

            def load_xsb(dst, r, sb, tag, eng, bufs):
                # hi/lo fp8 slab pair, [128, 4(cp), 2(c), 512]
                pair = {}
                for part in ("hi", "lo"):
                    t = xspool.tile([128, 4, 2, 512], fp8, tag=tag,
                                    name=tag + part, bufs=bufs)
                    eng.dma_start(t[:], r[part][:, :, bass.ts(sb, 512)])
                    pair[part] = t
                dst[sb] = pair

            # ---- weight tiles (w tag cycles 3 slots; wo reuses wk's) ----
            # layout [128, NPAIR, 2(part), 4(cp), 2(c), 128]
            wk_t = wpool.tile([128, NPAIR, 2, 4, 2, 128], fp8, tag="w",
                              name="wk_t")
            wq_t = wpool.tile([128, NPAIR, 2, 4, 2, 128], fp8, tag="w",
                              name="wq_t")
            wv_t = wpool.tile([128, 2, 4, 2, DL], fp8, tag="w", name="wv_t")
            wo_state = {}
            W_HI, W_LO = 0, 1

            # ---- prologue DMAs, ordered by first need ----
            xks0, xqs0 = {}, {}
            for part in ("hi", "lo"):
                xks0[part] = xspool.tile([128, 4, 2, 512], fp8, tag="xks",
                                         name="xks0" + part, bufs=8)
                xqs0[part] = xspool.tile([128, 4, 2, 512], fp8, tag="xqs",
                                         name="xqs0" + part, bufs=6)
            xk_sb[0], xq_sb[0] = xks0, xqs0
            nc.sync.dma_start(wk_t[:, 0, W_HI], wkd["hi"][0])
            nc.sync.dma_start(xks0["hi"][:], xkr["hi"][:, :, 0:512])
            nc.sync.dma_start(wq_t[:, 0, W_HI], wqd["hi"][0])
            nc.sync.dma_start(xqs0["hi"][:], xqr["hi"][:, :, 0:512])
            nc.sync.dma_start(bk_sb[:], bk2[:, :])
            nc.sync.dma_start(bq_sb[:], bq2[:, :])
            nc.sync.dma_start(wk_t[:, 0, W_LO], wkd["lo"][0])
            nc.sync.dma_start(xks0["lo"][:], xkr["lo"][:, :, 0:512])
            nc.sync.dma_start(wq_t[:, 0, W_LO], wqd["lo"][0])
            nc.sync.dma_start(xqs0["lo"][:], xqr["lo"][:, :, 0:512])
            load_xsb(xk_sb, xkr, 1, "xks", nc.sync, 8)
            load_xsb(xk_sb, xkr, 2, "xks", nc.sync, 8)
            load_xsb(xk_sb, xkr, 3, "xks", nc.sync, 8)
            nc.sync.dma_start(wv_t[:, W_HI], wvr["hi"][:])
            nc.sync.dma_start(wv_t[:, W_LO], wvr["lo"][:])
            load_xsb(xv_sb, xvr, 0, "xvs", nc.sync, 8)
            load_xsb(xq_sb, xqr, 1, "xqs", nc.sync, 6)
            load_xsb(xv_sb, xvr, 1, "xvs", nc.sync, 8)
            load_xsb(xv_sb, xvr, 2, "xvs", nc.sync, 8)
            load_xsb(xv_sb, xvr, 3, "xvs", nc.sync, 8)

            def L_xq(sb):
                def u():
                    # memset-gated, DVE-issued: can't be hoisted into the
                    # congested prologue wire by the list scheduler
                    pair = {}
                    for part in ("hi", "lo"):
                        t = xspool.tile([128, 4, 2, 512], fp8, tag="xqs",
                                        name="xqs" + part, bufs=6)
                        nc.vector.memset(t[0:1, 0, 0, 0:1], 0)
                        nc.sync.dma_start(
                            t[:], xqr[part][:, :, bass.ts(sb, 512)])
                        pair[part] = t
                    xq_sb[sb] = pair
                return u

            def L_w(p):
                def u():
                    for t, d in ((wk_t, wkd), (wq_t, wqd)):
                        for wp, part in ((W_HI, "hi"), (W_LO, "lo")):
                            nc.vector.memset(t[0:1, p, wp, 0, 0, 0:1], 0)
                            nc.sync.dma_start(t[:, p, wp], d[part][p])
                return u

            # ---- work units (emitted when popped) ----
            DR = mybir.MatmulPerfMode.DoubleRow

            def dr_proj_pass(pq, w_t, p, xs, wp, xp, first, last):
                for cp in range(4):
                    nc.tensor.matmul(
                        out=pq[:], lhsT=w_t[:, p, wp, cp],
                        rhs=xs[xp][:, cp],
                        start=(first and cp == 0),
                        stop=(last and cp == 3),
                        perf_mode=DR, skip_group_check=True,
                    )

            def dr_proj(pq, w_t, p, xs):
                # hi*hi + hi*lo + lo*hi chains, one psum group
                for i, (wp, xp) in enumerate(
                        ((W_HI, "hi"), (W_LO, "hi"), (W_HI, "lo"))):
                    dr_proj_pass(pq, w_t, p, xs, wp, xp, i == 0, i == 2)

            def qk_evac(dst, p, sb, pq, b_sb):
                # two lane-shifted fp8 copies: psum rows [0:64] = t_lo half,
                # rows [64:128] = t_hi half (host permutes W rows per pair)
                g, qq = p // 2, p % 2
                for thi in range(2):
                    nc.vector.tensor_scalar(
                        out=dst[64 * qq:64 * qq + 64, g, thi,
                                bass.ts(sb, 512)],
                        in0=pq[64 * thi:64 * thi + 64, :],
                        scalar1=1.0 / 64,
                        scalar2=b_sb[64 * thi:64 * thi + 64, p:p + 1],
                        op0=mybir.AluOpType.mult, op1=mybir.AluOpType.add,
                    )

            def KQ_unit(dst, w_t, b_sb, p, sb, xsb):
                # three ~0.43us quanta so scores/exps weave through the
                # in-order PE stream between them
                st = {}

                def q1():
                    st["pq"] = psum.tile([128, 512], f32, tag="gemm",
                                         name="pkq")
                    dr_proj_pass(st["pq"], w_t, p, xsb[sb], W_HI, "hi",
                                 True, False)

                def q2():
                    dr_proj_pass(st["pq"], w_t, p, xsb[sb], W_LO, "hi",
                                 False, False)

                def q3():
                    dr_proj_pass(st["pq"], w_t, p, xsb[sb], W_HI, "lo",
                                 False, True)
                    qk_evac(dst, p, sb, st["pq"], b_sb)
                return [q1, q2, q3]

            def K_unit(p, sb):
                return KQ_unit(kT, wk_t, bk_sb, p, sb, xk_sb)

            def Q_unit(p, sb):
                return KQ_unit(qT, wq_t, bq_sb, p, sb, xq_sb)

            def V_unit(p, jb):
                def u():
                    pv = psum.tile([128, 2, 64], f32, tag="gemm", name="pv")
                    xs = xv_sb[jb // 4]
                    first = True
                    for xp, wp in (("hi", W_HI), ("lo", W_HI), ("hi", W_LO)):
                        for cp in range(4):
                            nc.tensor.matmul(
                                out=pv[:],
                                lhsT=xs[xp][:, cp, :, bass.ts(jb % 4, 128)],
                                rhs=wv_t[:, wp, cp, :,
                                         2 * p * 64:(2 * p + 2) * 64],
                                start=first,
                                stop=(wp == W_LO and cp == 3),
                                perf_mode=DR, skip_group_check=True,
                            )
                            first = False
                    nc.vector.tensor_scalar_mul(
                        out=v_sb[:, jb, 2 * p:2 * p + 2, 0:64], in0=pv[:],
                        scalar1=1.0 / 64)
                return u

            def wo_unit():
                def u():
                    wo_sb = wpool.tile([128, NPAIR, D], bf16, tag="w",
                                       name="wo_sb")
                    nc.vector.memset(wo_sb[0:1, 0, 0:1], 0)
                    nc.sync.dma_start(
                        wo_sb[:], wo.rearrange("(p d) e -> d p e", d=128))
                    wo_state["wo"] = wo_sb
                return u

            def OP_half(row, e, st, split=False):
                # one e-half of an outproj row, in two PE quanta
                def u1():
                    wo_sb = wo_state["wo"]
                    if e == 0:
                        st["ot"] = opool.tile([128, 2, 512], bf16, tag="ot",
                                              name="ot")
                    st["po"] = psum.tile([128, 512], f32, tag="gemm",
                                         name="po")
                    for p in range(2):
                        nc.tensor.matmul(
                            out=st["po"][:],
                            lhsT=zT[:, p, bass.ts(row, 128)],
                            rhs=wo_state["wo"][:, p, bass.ts(e, 512)],
                            start=(p == 0), stop=False,
                        )

                def u2():
                    ot, po = st["ot"], st["po"]
                    wo_sb = wo_state["wo"]
                    for p in range(2, NPAIR):
                        nc.tensor.matmul(
                            out=po[:],
                            lhsT=zT[:, p, bass.ts(row, 128)],
                            rhs=wo_sb[:, p, bass.ts(e, 512)],
                            start=False, stop=(p == NPAIR - 1),
                        )
                    nc.vector.tensor_copy(out=ot[:, e, :], in_=po[:])
                    if split:
                        nc.sync.dma_start(
                            out[bass.ts(row, 128), bass.ts(e, 512)],
                            ot[:, e, :])
                    elif e == 1:
                        nc.sync.dma_start(
                            out[bass.ts(row, 128), :], ot[:])
                return [u1, u2]

            def OP_unit(row, split=False):
                st = {}
                return (OP_half(row, 0, st, split) +
                        OP_half(row, 1, st, split))

            # ---- attention ----
            def score_exp(p, ib, jb, engs="AA"):
                # per-head [128, 512] sc tiles: 4 one-bank rotation slots
                # (tags sc0/sc1 x 2 bufs) so exp handoff latency never gates
                # the next score matmul; one exp instruction per head
                g, qq = p // 2, p % 2
                pair = []
                for h in range(2):
                    P0 = 64 * qq + 32 * h
                    sc = psum.tile([128, 512], f32, tag=f"sc{h}",
                                   name=f"sc{h}")
                    nc.tensor.matmul(
                        out=sc[:],
                        lhsT=kT[P0:P0 + 32, g, :, bass.ts(jb, 128)],
                        rhs=qT[P0:P0 + 32, g, :, bass.ts(ib, 512)],
                        start=True, stop=True,
                        perf_mode=DR,
                        tile_position=(P0, 0),
                    )
                    et = epool.tile([128, 512], bf16, tag="et",
                                    name=f"et{h}")
                    if engs[h] == "A":
                        nc.scalar.activation(
                            out=et[:], in_=sc[:],
                            func=mybir.ActivationFunctionType.Exp,
                            scale=0.125,
                        )
                    else:
                        e = nc.vector if engs[h] == "D" else nc.gpsimd
                        e.tensor_scalar(
                            out=et[:].bitcast(i16), in0=sc[:],
                            scalar1=EXP_A, scalar2=EXP_B,
                            op0=mybir.AluOpType.mult,
                            op1=mybir.AluOpType.add,
                        )
                    pair.append(et)
                return pair

            def ctx_jb(p, ctx_t, ets, jb):
                # 8 accumulating matmuls: out[i, t+den] per (h, ipart).
                # Both heads' groups span the whole jb range (start on the
                # first sub-tile touch, stop on the last); disjoint [128,65]
                # sub-tiles of one bank are first-touch-zeroed by the group.
                for h in range(2):
                    for ip in range(4):
                        nc.tensor.matmul(
                            out=ctx_t[h][:, ip, :],
                            lhsT=ets[jb][h][:, bass.ts(ip, 128)],
                            rhs=v_sb[:, jb, 2 * p + h, 0:65],
                            start=(jb == 0 and ip == 0),
                            stop=(jb == NJB - 1 and ip == 3),
                            skip_group_check=True,
                        )

            def evac_tr(p, ib, ctx_t, z_t):
                # one normalize per head: denominator broadcast along t
                for h in range(2):
                    den = zpool.tile([128, 4, 1], f32, tag="den", bufs=4)
                    nc.vector.reciprocal(out=den[:],
                                         in_=ctx_t[h][:, :, 64:65])
                    nc.vector.tensor_tensor(
                        out=z_t[:, :, 64 * h:64 * h + 64],
                        in0=ctx_t[h][:, :, 0:64],
                        in1=den[:].broadcast_to([128, 4, 64]),
                        op=mybir.AluOpType.mult,
                    )
                for ip in range(4):
                    transp(p, ib, z_t, ip)

            def transp(p, ib, z_t, ip):
                nc.sync.dma_start(
                    zT[:, p, ib * 512 + ip * 128:ib * 512 + (ip + 1) * 128],
                    z_t[:, ip, :], transpose=True)

            # exp engines per (jb, head) slot: h0 on Act, h1 rotates DVE/Act
            # -> 21 Act / 11 DVE head-tiles per block.  (GPSIMD cannot read
            # PSUM, so Pool gets no exps.)
            ENG_PAT = ["AD", "AD", "AA", "AD", "AD", "AA", "AD", "AD",
                       "AA", "AD", "AD", "AA", "AD", "AD", "AA", "AD"]
            ENG_PAT_TAIL = ["AD", "AA", "AD", "AA", "AD", "AA", "AD", "AA",
                            "AD", "AA", "AD", "AA", "AD", "AA", "AD", "AA"]

            # ---- schedule: per-block interleaved work units ----
            def WU(n):
                def u():
                    warm(n)
                return u

            KU, QU, VU = K_unit, Q_unit, V_unit
            sched = [[] for _ in range(17)]
            loads = [[] for _ in range(17)]
            loads[1] = [L_w(1), L_xq(2), L_w(2)]
            loads[2] = [L_w(3), L_xq(3)]
            loads[10] = [wo_unit()]
            sched[0] = [WU(6), WU(6), KU(0, 1), WU(6), KU(0, 2),
                        WU(6), KU(0, 3), VU(0, 0), VU(0, 1),
                        VU(0, 2), VU(0, 3), VU(0, 4),
                        VU(0, 5), VU(0, 6), VU(0, 7), QU(0, 1)]
            sched[1] = [QU(0, 2), QU(1, 0),
                        QU(2, 0)]
            sched[2] = [QU(3, 0), KU(1, 0), QU(0, 3),
                        VU(1, 0), VU(1, 1), VU(1, 2), VU(1, 3), VU(1, 4),
                        VU(1, 5), VU(1, 6)]
            sched[3] = [KU(1, 1), KU(1, 2), VU(1, 7), VU(1, 8), VU(1, 9),
                        VU(1, 10), VU(1, 11), VU(1, 12)]
            sched[4] = [KU(1, 3), VU(1, 13), VU(1, 14), VU(1, 15),
                        QU(1, 1)]
            sched[5] = [QU(1, 2), KU(2, 0),
                        VU(2, 0), VU(2, 1), VU(2, 2)]
            sched[6] = [QU(1, 3), KU(2, 1),
                        VU(2, 3), VU(2, 4), VU(2, 5), VU(2, 6)]
            sched[7] = [KU(2, 2), KU(2, 3),
                        VU(2, 7), VU(2, 8), VU(2, 9), VU(2, 10)]
            sched[8] = [VU(2, 11), VU(2, 12), VU(2, 13), VU(2, 14),
                        VU(2, 15), QU(2, 1)]
            sched[9] = [QU(2, 2), KU(3, 0),
                        VU(3, 0), VU(3, 1), VU(3, 2)]
            sched[10] = [QU(2, 3), KU(3, 1),
                         VU(3, 3), VU(3, 4), VU(3, 5)]
            sched[11] = [KU(3, 2), KU(3, 3),
                         VU(3, 6), VU(3, 7), VU(3, 8), VU(3, 9)]
            sched[12] = [VU(3, 10), VU(3, 11), VU(3, 12), VU(3, 13),
                         VU(3, 14), VU(3, 15), QU(3, 1)]
            sched[13] = ([QU(3, 2)] +
                         [u for sub in range(4) for u in OP_unit(0 * 4 + sub)])
            sched[14] = ([QU(3, 3)] +
                         [u for sub in range(4) for u in OP_unit(1 * 4 + sub)])
            sched[15] = [u for sub in range(4) for u in OP_unit(2 * 4 + sub)]
            sched[16] = (OP_unit(12) + OP_unit(13) +
                         OP_unit(14, split=True) + OP_unit(15, split=True))

            def flat(lst):
                o = []
                for u in lst:
                    o.extend(u) if isinstance(u, list) else o.append(u)
                return o

            sched = [flat(s) for s in sched]

            # ---- prologue compute ----
            warm(20)
            kq1, kq2, kq3 = K_unit(0, 0)
            qq1, qq2, qq3 = Q_unit(0, 0)
            kq1(); qq1(); kq2(); qq2(); kq3(); qq3()

            carry = []
            for blk in range(16):
                p, ib = blk // 4, blk % 4
                LAG = 8 if blk == 0 else 6
                ctx_t = [psum.tile([128, 4, 65], f32, tag="ctx",
                                   name=f"ctx{h}") for h in range(2)]
                z_t = zpool.tile([128, 4, 128], bf16, tag="zsb", name="z_t",
                                 bufs=2)
                ets = []
                if blk == 1:
                    # weave leftover V(0,8..15) ahead of the carried ctx
                    # units so each ctx_jb(j) sees v_sb(j) already emitted
                    post = []
                    for u in loads[blk]:
                        u()
                    for i in range(8, 16):
                        post.append(VU(0, i))
                        post.append(carry[i - 8])
                    post += list(carry[8:]) + sched[blk]
                else:
                    post = carry + sched[blk]
                for u in loads[blk]:
                    u()
                popped = 0

                for jj in range(0, NJB, 2):
                    # paired emission: fill both sc bufs, then two slots of
                    # PE work while the exp engines drain them.  Pops come
                    # before the ctx emissions: carried units include the
                    # V_units whose v_sb data those ctx matmuls consume.
                    pat = ENG_PAT_TAIL if blk >= 13 else ENG_PAT
                    ets.append(score_exp(p, ib, jj, pat[jj]))
                    ets.append(score_exp(p, ib, jj + 1, pat[jj + 1]))
                    want = (len(post) * (jj + 2)) // NJB
                    while popped < want:
                        post[popped]()
                        popped += 1
                    if jj >= LAG:
                        ctx_jb(p, ctx_t, ets, jj - LAG)
                    if jj + 1 >= LAG:
                        ctx_jb(p, ctx_t, ets, jj + 1 - LAG)
                while popped < len(post):
                    post[popped]()
                    popped += 1

                def mk_tail(p=p, ib=ib, ctx_t=ctx_t, z_t=z_t, ets=ets,
                            LAG=LAG):
                    def t_ctx(j):
                        return lambda: ctx_jb(p, ctx_t, ets, j)
                    def t3():
                        evac_tr(p, ib, ctx_t, z_t)
                    def t4():
                        pass
                    return [t_ctx(j) for j in range(NJB - LAG, NJB)] + \
                        [t3, t4]

                carry = mk_tail()

            for u in carry[:-2]:
                u()
            carry[-2]()
            warm(18, big=True)
            carry[-1]()
            for u in sched[16]:
                u()

    nc.compile()
    return nc


def _split8(a):
    """f32 array -> (hi, lo) fp8e4m3 pair with a ~= hi + lo."""
    a = np.asarray(a, np.float32)
    hi = a.astype(F8)
    lo = (a - hi.astype(np.float32)).astype(F8)
    return hi, lo


def _prep_inputs(Q, K, V, Wq, bq, Wk, bk, Wv, bv, Wo, bo):
    """Build the 8 per-core input maps (host-side shard + transpose + cast).

    x and the qkv weights ship as fp8e4m3 hi/lo residual pairs (weights
    pre-scaled by 64 so the lo parts stay in fp8's normal range; the kernel
    rescales by 1/64 in the bias-add)."""
    xt = {}  # (name, batch) -> (hi, lo) [D, S] fp8
    for nm, full in (("xq", Q), ("xk", K), ("xv", V)):
        for b in range(B):
            xt[(nm, b)] = _split8(np.ascontiguousarray(full[b].T))

    def w_half(W, h0):
        # W [H,T,D] -> [D, HL*T] (scaled by 64 for the fp8 split)
        w = W[h0:h0 + HL]                       # [HL,T,D]
        w = w.transpose(2, 0, 1).reshape(D, DL)  # [D, HL*T]
        return np.ascontiguousarray(w) * np.float32(64)

    def w_half_pm(W, h0):
        # pair-major chunked: [NPAIR, 128(r), NKC(c), 128(t)], pair-local
        # t rows permuted for the fp8-DR evac split
        w = w_half(W, h0)                        # [D, DL] f32 (x64)
        w4 = w.reshape(NKC, 128, NPAIR, 128)     # [c, r, p, t]
        w4 = w4[..., T_PERM]
        return _split8(np.ascontiguousarray(w4.transpose(2, 1, 0, 3)))

    def b_half(bias, h0):
        return np.ascontiguousarray(
            bias[h0:h0 + HL].reshape(NPAIR, 128)[:, T_PERM].T
        ).astype(np.float32)

    in_maps = []
    for c in range(NC):
        b, half = c // 2, c % 2
        h0 = half * HL
        off = half * DL
        wo_c = np.ascontiguousarray(Wo[:, off:off + DL].T).astype(BF16)
        wqs, wks = w_half_pm(Wq, h0), w_half_pm(Wk, h0)
        wvs = _split8(w_half(Wv, h0))
        m = {
            "bq2": b_half(bq, h0), "bk2": b_half(bk, h0),
            "wo": wo_c,
        }
        for i, part in enumerate(("hi", "lo")):
            m[f"xq_{part}"] = xt[("xq", b)][i]
            m[f"xk_{part}"] = xt[("xk", b)][i]
            m[f"xv_{part}"] = xt[("xv", b)][i]
            m[f"wq_{part}"] = wqs[i]
            m[f"wk_{part}"] = wks[i]
            m[f"wv_{part}"] = wvs[i]
        in_maps.append(m)
    return in_maps


def _combine(results, bv, Wo, bo):
    const = (Wo.astype(np.float64) @ bv.reshape(-1).astype(np.float64)
             + bo.astype(np.float64)).astype(np.float32)
    out = np.empty((B, S, D), np.float32)
    for b in range(B):
        out[b] = (results[2 * b]["out"].astype(np.float32)
                  + results[2 * b + 1]["out"].astype(np.float32) + const)
    return out


def _get_runner(nc):
    """Persistent jitted shard_map runner (run_bass_via_pjrt re-traces per
    call; this builds the identical execution once and reuses it)."""
    if "runner" in _STATE:
        return _STATE["runner"]
    import jax
    from jax.experimental.shard_map import shard_map
    from jax.sharding import Mesh, PartitionSpec
    import concourse.bass2jax as B2
    B2.install_neuronx_cc_hook()
    pname = nc.partition_id_tensor.name if nc.partition_id_tensor else None
    in_names, out_names, out_avals, zshapes = [], [], [], []
    for alloc in nc.m.functions[0].allocations:
        if not isinstance(alloc, mybir.MemoryLocationSet):
            continue
        name = alloc.memorylocations[0].name
        if alloc.kind == "ExternalInput":
            if name != pname:
                in_names.append(name)
        elif alloc.kind == "ExternalOutput":
            out_names.append(name)
            shape = tuple(alloc.tensor_shape)
            dtype = mybir.dt.np(alloc.dtype)
            out_avals.append(jax.core.ShapedArray(shape, dtype))
            zshapes.append((shape, dtype))
    n_params, n_outs = len(in_names), len(out_names)
    all_in = list(in_names) + list(out_names)
    if pname is not None:
        all_in.append(pname)
    donate = tuple(range(n_params, n_params + n_outs))

    def _body(*args):
        operands = list(args)
        if pname is not None:
            operands.append(B2.partition_id_tensor())
        outs = B2._bass_exec_p.bind(
            *operands, out_avals=tuple(out_avals), in_names=tuple(all_in),
            out_names=tuple(out_names), lowering_input_output_aliases=(),
            sim_require_finite=True, sim_require_nnan=True, nc=nc)
        return tuple(outs)

    devices = jax.devices()[:NC]
    mesh = Mesh(np.asarray(devices), ("core",))
    sharded = jax.jit(
        shard_map(_body, mesh=mesh,
                  in_specs=(PartitionSpec("core"),) * (n_params + n_outs),
                  out_specs=(PartitionSpec("core"),) * n_outs,
                  check_rep=False),
        donate_argnums=donate, keep_unused=True)
    _STATE["runner"] = (sharded, in_names, out_names, zshapes)
    return _STATE["runner"]


def kernel(Q, K, V, Wq, bq, Wk, bk, Wv, bv, Wo, bo, _trace=False):
    args = [np.asarray(a, np.float32) for a in
            (Q, K, V, Wq, bq, Wk, bk, Wv, bv, Wo, bo)]
    Q, K, V, Wq, bq, Wk, bk, Wv, bv, Wo, bo = args
    if "nc" not in _STATE:
        _STATE["nc"] = _build()
    nc = _STATE["nc"]
    in_maps = _prep_inputs(Q, K, V, Wq, bq, Wk, bk, Wv, bv, Wo, bo)
    try:
        sharded, pnames, out_names, zshapes = _get_runner(nc)
        concat_in = [np.concatenate([m[name] for m in in_maps], axis=0)
                     for name in pnames]
        concat_zeros = [np.zeros((NC * sh[0], *sh[1:]), dt)
                        for sh, dt in zshapes]
        arrs = [np.asarray(a) for a in sharded(*concat_in, *concat_zeros)]
        results = [
            {name: arrs[i][c * zshapes[i][0][0]:(c + 1) * zshapes[i][0][0]]
             for i, name in enumerate(out_names)}
            for c in range(NC)
        ]
    except Exception:
        res = run_bass_kernel_spmd(nc, in_maps, list(range(NC)), trace=False)
        results = res.results
    return _combine(results, bv, Wo, bo)

